# revision 21
# baseline (speedup 1.0000x reference)
"""Distributed causal multi-head attention for 8 TRN2 NeuronCores.

Problem: x[4,2048,1024], per-head Q/K/V [16,64,1024], O [1024,1024].
  q,k,v = per-head projections of x; scores = q@k^T (no 1/sqrt(d));
  causal softmax; z = attn@v; out = z @ O^T.

Sharding (head-parallel): core j owns heads {2j, 2j+1} for ALL batches.
Per core:
  - x/Wq/Wk in fp16 (10-bit mantissa): scores are ~N(0, 64) with no 1/sqrt(d)
    scaling, so exp() amplifies absolute score error; bf16 inputs would give
    ~4% output error while fp16 gives ~0.5% and runs at full PE rate
    (f32r runs at half rate; f32 at quarter rate).
  - scoresT [k, q] layout: the softmax denominator comes for free from a
    ones-column appended to the PV stationary operand (psum row 64 = l);
    exp runs on ACT from 2-bank psum groups, psum -> sbuf bf16.
  - causal mask applied post-exp by multiplying with one of two STATIC mask
    tiles (built once at startup with gpsimd affine_select) as a single DVE
    tensor_mul per (head, diagonal k-pair). gpsimd-resident affine_select
    was ~2x slower per element and its queue backlog (broadcast/collective/
    out-writes) stalled PV by 1.5-4.5us per unit.
  - z is exchanged via FOUR AllToAlls. Measured collective cost model: a
    "cold" collective pays ~12.5us dispatch between its gpsimd trigger and
    the CC transfer (back-to-back collectives dispatch in ~2us), plus peer
    rendezvous equal to core arrival skew, plus ~1us/100KB transfer. Also
    z-write DMA completion gates the trigger, so z-write descriptor size
    matters (64-col routing = 128B descriptors measured ~5us per [64,512]
    write; full-unit slots give 1KB descriptors at ~1us). Hence:
      A2A_1: all 8 phase-I units {1,3}, slot = one full unit (512 cols,
             1KB descs), fired at the phase boundary; its 4 O-proj chunks
             land mid phase II.
      A2A_2: the 4 big phase-II units {(b,2)}, slot = half unit (256 cols,
             512B descs), fired right after (3,2)'s deferred tail, mid
             phase II; its 2 chunks land at the tail start.
      A2A_3/4: the tiny {(b,0)} units in pairs (slot 128 cols), fired as
             late pipeline stages; their latency is hidden under the 4
             deferred chunks of A2A_2/3 that fill the PE at the tail.
    Output rows are resharded accordingly (core j owns a W-wide column
    slice of each unit, W = 512/#cores-per-unit); kernel() unshards.
  - phase I = all projections + attention {1,3} (PE-bound: proj has no exp
    work to hide); phase II = {(0,2),(1,2),(2,2),(3,2),(0,0),(1,0),(2,0),
    (3,0)} - exp-heavy units first so the tail rides on the small ones.
    O-proj chunks are interleaved where the PE has slack, pinned by
    add_dep_helper anchors (the Tile scheduler otherwise hoists their
    zrecv-gated stationary loads ahead of attention on the in-order PE
    queue, stalling it for a whole collective - measured 49us).
  - zrecv reads are data-gated on a collective's completion semaphore and
    BLOCK the sync queue while waiting, delaying the z writes queued
    behind them (and hence the next collective): every zrecv is issued
    only where its collective is already (nearly) complete.
  - x feed: scalar+gpsimd queues; sync joins only for batch-0 units (it
    must stay clear of bulk traffic once softmax-gated z writes exist).
    wqk/x tile pairs interleave at startup so the first proj matmul needs
    only the first pair; a dummy exp warms the ACT table during the DMA
    wait (first real exp otherwise pays ~4.5us table load + ramp).
  - measured hazards baked in: ACT ops cost ~700ns fixed (never split exps);
    sub-128-row quadrant matmuls run ~1.5x slower (keep kz zero-padded);
    within a unit, scores(g+1) is emitted before PV(g) so the in-order PE
    never waits on exp(g); each unit's final PV + normalize are deferred
    into the next proj/unit.
"""

import os

import numpy as np
import ml_dtypes

import concourse.mybir as mybir
import concourse.tile as tile
from concourse.tile import add_dep_helper
from concourse import bacc
from concourse.bass_utils import run_bass_kernel_spmd

BF16 = mybir.dt.bfloat16
F32 = mybir.dt.float32
F32R = mybir.dt.float32r
FP16 = mybir.dt.float16

B, M, NH, DH = 4, 1024, 16, 64
NCORES = 8

# AllToAll groups: (units in completion order, slot width, out-row base).
# Slot width W = 512 * len(units) / 8; core j owns cols [W*(j%cpu), +W) of
# unit units[j//cpu] where cpu = 512//W cores share a unit.
A2AS = [
    ([(0, 1), (0, 3), (1, 1), (1, 3), (2, 1), (2, 3), (3, 1), (3, 3)], 512, 0),
    ([(0, 2), (1, 2), (2, 2), (3, 2)], 256, 512),
    ([(0, 0), (1, 0), (2, 0), (3, 0)], 256, 768),
]
UNIT_SLOT = {}
for _ai, (_units, _w, _base) in enumerate(A2AS):
    for _u, _bm in enumerate(_units):
        UNIT_SLOT[_bm] = (_ai, _u)

LAST_EXEC_TIME_NS = None


def build(S=2048):
    GQ = B * S
    CH = GQ // NCORES      # output rows per core

    nc = bacc.Bacc("TRN2", target_bir_lowering=False, debug=False, num_devices=NCORES)
    xt_ext = nc.dram_tensor("xt", [B, M, S], FP16, kind="ExternalInput")
    wqk_ext = nc.dram_tensor("wqk", [M, 256], FP16, kind="ExternalInput")
    wv_ext = nc.dram_tensor("wv", [M, 128], FP16, kind="ExternalInput")
    ot_ext = nc.dram_tensor("ot", [M, M], BF16, kind="ExternalInput")
    # fp16 output (~5e-4 rounding, well within budget) halves the tail
    # out-write traffic; kernel() casts back to f32.
    out_ext = nc.dram_tensor("out", [CH, M], FP16, kind="ExternalOutput")

    Exp = mybir.ActivationFunctionType.Exp

    with (
        tile.TileContext(nc) as tc,
        tc.tile_pool(name="wpool", bufs=1) as wpool,
        tc.tile_pool(name="xt", bufs=32) as xt_pool,
        tc.tile_pool(name="qk", bufs=1) as qk_pool,
        tc.tile_pool(name="kz", bufs=1) as kz_pool,
        tc.tile_pool(name="vp", bufs=1) as v_pool,
        tc.tile_pool(name="ep", bufs=7) as e_pool,
        tc.tile_pool(name="zp", bufs=12) as z_pool,
        tc.tile_pool(name="zr", bufs=5) as zr_pool,
        tc.tile_pool(name="ob", bufs=2) as ob_pool,
        tc.tile_pool(name="nrm", bufs=2) as nrm_pool,
        tc.tile_pool(name="ps_sc", bufs=2, space="PSUM") as ps_sc,
        tc.tile_pool(name="ps_z", bufs=1, space="PSUM") as ps_z,
        tc.tile_pool(name="ps_gen", bufs=2, space="PSUM") as ps_gen,
        tc.tile_pool(name="dram", bufs=1, space="DRAM") as dram,
    ):
        xq3 = [nc.scalar, nc.gpsimd, nc.sync]
        xq2 = [nc.scalar, nc.gpsimd]
        wqk_sb, wv_sb, ot_sb = [], [], []
        for m in range(8):
            wqk_sb.append(wpool.tile([128, 256], FP16, name=f"wqk{m}", tag=f"wqk{m}"))
            wv_sb.append(wpool.tile([128, 128], FP16, name=f"wv{m}", tag=f"wv{m}"))
            ot_sb.append(wpool.tile([128, 1024], BF16, name=f"ot{m}", tag=f"ot{m}"))

        a2a_in = [
            dram.tile([NCORES, 128, w], BF16, name=f"a2a_in{ai}")
            for ai, (_, w, _b) in enumerate(A2AS)
        ]
        a2a_out = [
            dram.tile([NCORES, 128, w], BF16, name=f"a2a_out{ai}")
            for ai, (_, w, _b) in enumerate(A2AS)
        ]

        # static causal masks for the two diagonal k-tile pairs of any unit:
        # mask[kpos, 512*kk + q] = 1 if q >= kpos + 128*(d0+kk) else 0.
        # Built AFTER the startup x DMAs are issued (emit_masks call below):
        # the memset/affine ops share the gpsimd queue with x-tile issues and
        # cost ~3.5us of first-tile delay if emitted first.
        masks = {}

        def emit_masks():
            for d0 in (0, 2):
                t = wpool.tile([128, 1024], BF16, name=f"mask{d0}", tag=f"mask{d0}")
                nc.gpsimd.memset(t[:], 1.0)
                for kk in range(2):
                    sl = t[:, 512 * kk:512 * (kk + 1)]
                    nc.gpsimd.affine_select(
                        out=sl,
                        in_=sl,
                        compare_op=mybir.AluOpType.is_ge,
                        fill=0.0,
                        base=-128 * (d0 + kk),
                        pattern=[[1, 512]],
                        channel_multiplier=-1,
                    )
                masks[d0] = t

        qk_sb = {}   # (ct, b, mq) -> [128, 512] fp16; ct0 = qT (2 heads)
        kz_sb = {}   # (h, b, mk) -> [128, 512] fp16 zero-padded per-head kT
        v_sb = {}    # (b, k_tile) -> [128, 130] bf16: [vA(64) | 1 | vB(64) | 1]
        zrt = {}     # (ai, c) -> [128, 1024] bf16 zrecv tile

        def emit_xt(b, mq, three_way=False):
            qs = xq3 if three_way else xq2
            xts = []
            for m in range(8):
                t = xt_pool.tile([128, 512], FP16, name="xtc")
                qs[m % len(qs)].dma_start(
                    t[:], xt_ext[b, 128 * m:128 * (m + 1), 512 * mq:512 * (mq + 1)]
                )
                xts.append(t)
            return xts

        def emit_startup():
            # wqk/x pairs: queue m%3 gets wqk_m then x(0,0)_m back to back, so
            # the m=0 pair (all the first matmul needs) lands first on scalar;
            # a zeroed dummy exp loads the ACT table during the DMA wait.
            xts0 = []
            for m in range(8):
                q = xq3[m % 3]
                q.dma_start(wqk_sb[m][:], wqk_ext[128 * m:128 * (m + 1), :])
                t = xt_pool.tile([128, 512], FP16, name="xtc")
                q.dma_start(t[:], xt_ext[0, 128 * m:128 * (m + 1), 0:512])
                xts0.append(t)
            warm = wpool.tile([1, 16], F32, name="actwarm", tag="actwarm")
            nc.gpsimd.memset(warm[:], 0.0)
            nc.scalar.activation(warm[:], warm[:], Exp)
            for m in range(8):
                xq3[m % 3].dma_start(wv_sb[m][:], wv_ext[128 * m:128 * (m + 1), :])
            return xts0

        def emit_proj(b, mq, xts, fin=None):
            for ct in range(2):
                ps = ps_gen.tile([128, 512], F32, name="psqk", tag="gen")
                for m in range(8):
                    nc.tensor.matmul(
                        ps[:],
                        wqk_sb[m][:, 128 * ct:128 * (ct + 1)],
                        xts[m][:],
                        start=(m == 0),
                        stop=(m == 7),
                    )
                if ct == 0:
                    t = qk_pool.tile(
                        [128, 512], FP16, name=f"qk{ct}_{b}_{mq}", tag=f"qk{ct}_{b}_{mq}"
                    )
                    nc.vector.tensor_copy(t[:], ps[:])
                    qk_sb[(ct, b, mq)] = t
                    # the previous attention unit's deferred tail (final PV +
                    # normalize) lands here: the ct0 matmuls above fill the
                    # PE while that unit's last exp drains on ACT
                    if fin is not None:
                        fin()
                else:
                    # kT is consumed only as zero-padded per-head copies:
                    # K=128 scores matmuls run at full rate while 64-row
                    # quadrant matmuls measured ~1.5x slower per instruction.
                    # Copies on DVE: ACT is the attention pacer (exp), keep
                    # it exp-only.
                    for h in range(2):
                        kz = kz_pool.tile(
                            [128, 512], FP16, name=f"kz{h}_{b}_{mq}",
                            tag=f"kz{h}_{b}_{mq}",
                        )
                        nc.vector.memset(kz[64 - 64 * h:128 - 64 * h, :], 0.0)
                        nc.vector.tensor_copy(
                            kz[64 * h:64 * (h + 1), :],
                            ps[64 * h:64 * (h + 1), :],
                        )
                        kz_sb[(h, b, mq)] = kz
            for stl in range(4):
                ps = ps_gen.tile([128, 128], F32, name="psv", tag="gen")
                for m in range(8):
                    nc.tensor.matmul(
                        ps[:],
                        xts[m][:, 128 * stl:128 * (stl + 1)],
                        wv_sb[m][:],
                        start=(m == 0),
                        stop=(m == 7),
                    )
                kt = 4 * mq + stl
                # layout [vA(64) | 1 | vB(64) | 1]: ones column makes PV row 64
                # the softmax denominator; z dims land on rows 0..63 (DVE
                # partition ranges must start at 0/32/64/96, so z-rows-first)
                vt = v_pool.tile([128, 130], BF16, name=f"v_{b}_{kt}", tag=f"v_{b}_{kt}")
                nc.gpsimd.memset(vt[:, 64:65], 1.0)
                nc.gpsimd.memset(vt[:, 129:130], 1.0)
                nc.vector.tensor_copy(
                    vt[:].rearrange("p (g c) -> p g c", g=2)[:, :, 0:64],
                    ps[:].rearrange("p (g c) -> p g c", g=2),
                )
                v_sb[(b, kt)] = vt

        def emit_attn(b, mq, fin_prev=None, defer=False):
            nk = 4 * (mq + 1)
            # pz is allocated lazily at the first PV so pool-slot WAR
            # tracking stays consistent with deferred tails (the previous
            # unit's final PV may be emitted after this unit starts)
            pzc = []

            def get_pz():
                if not pzc:
                    pzc.append(ps_z.tile([128, 1024], F32, name="pz", tag="pz"))
                return pzc[0]

            def emit_scores_exp(g):
                # per head: 2 scores matmuls then immediately the exp, so ACT
                # starts on head 0 while the PE does head 1's scores. One
                # full-width exp per (group, head): ACT instructions have
                # ~700ns fixed overhead, so fewer/wider beats masked-region
                # skipping (measured +24us ACT when split per ktile).
                es = []
                for h in range(2):
                    psc = ps_sc.tile([128, 1024], F32, name="psc", tag="sc")
                    for kk in range(2):
                        kt = 2 * g + kk
                        mk, ktl = kt // 4, kt % 4
                        # diagonal tiles: q columns < 128*d are fully masked,
                        # so skip them in both scores and PV. The psum left
                        # unwritten holds stale-but-finite values whose exp
                        # is zeroed by the mask multiply (q < 128d => masked).
                        d = kt - 4 * mq
                        q0 = 128 * d if d > 0 else 0
                        nc.tensor.matmul(
                            psc[:, 512 * kk + q0:512 * (kk + 1)],
                            kz_sb[(h, b, mk)][:, 128 * ktl:128 * (ktl + 1)],
                            qk_sb[(0, b, mq)][:, q0:512],
                            start=True,
                            stop=True,
                        )
                    e = e_pool.tile([128, 1024], BF16, name="etile")
                    nc.scalar.activation(e[:], psc[:], Exp)
                    es.append(e)
                d0 = 2 * g - 4 * mq
                if d0 >= 0:  # diagonal pair: zero where k + 128*d > q
                    for h in range(2):
                        nc.vector.tensor_mul(es[h][:], es[h][:], masks[d0][:])
                return es

            last_pe = [None]

            def emit_pv(g, es):
                pz = get_pz()
                for kk in range(2):
                    kt = 2 * g + kk
                    d = kt - 4 * mq
                    q0 = 128 * d if d > 0 else 0
                    vt = v_sb[(b, kt)]
                    for h in range(2):
                        last_pe[0] = nc.tensor.matmul(
                            pz[0:65, 512 * h + q0:512 * h + 512],
                            vt[:, 65 * h:65 * h + 65],
                            es[h][:, 512 * kk + q0:512 * (kk + 1)],
                            start=(kt == 0),
                            stop=(kt == nk - 1),
                        )

            # software pipeline: scores(g+1) is emitted BEFORE PV(g). The PE
            # queue is in-order, so otherwise PV(g)'s wait on exp(g) blocks
            # scores(g+1) that could already run - ~1us/group of PE idle.
            # The final PV + normalize are deferred (fin) so the NEXT
            # proj/unit's first matmuls can fill the last exp's latency; the
            # previous unit's fin lands right after this unit's first group.
            prev = None
            first = True
            for g in range(nk // 2):
                es = emit_scores_exp(g)
                if first:
                    if fin_prev is not None:
                        fin_prev()
                    first = False
                if prev is not None:
                    emit_pv(prev[0], prev[1])
                prev = (g, es)

            def fin():
                emit_tail(prev)
                return last_pe[0]

            def emit_tail(prev):
                emit_pv(prev[0], prev[1])
                pz = get_pz()
                # normalize: pz row 64 of each half = l. partition_broadcast
                # only reads from base partition 0, and DVE can't shift
                # partitions, so DMA the l row from psum partition 64 to
                # sbuf partition 0 first. The l row is copied FIRST (tiny DVE
                # op) so the l0-dma/reciprocal/broadcast chain starts ~1.5us
                # before the bulk z copy lands - this chain gates the z
                # writes and hence the collective triggers.
                zcp = nrm_pool.tile([65, 1024], F32, name="zcp", tag="zcp")
                nc.vector.tensor_copy(zcp[64:65, :], pz[64:65, :])
                l0 = nrm_pool.tile([1, 1024], F32, name="l0", tag="l0")
                nc.gpsimd.dma_start(l0[:], zcp[64:65, :])
                nc.vector.tensor_copy(zcp[0:64, 0:512], pz[0:64, 0:512])
                nc.scalar.activation(
                    zcp[0:64, 512:1024],
                    pz[0:64, 512:1024],
                    mybir.ActivationFunctionType.Copy,
                )
                rec = nrm_pool.tile([1, 1024], F32, name="rec", tag="rec")
                nc.vector.reciprocal_approx_fast(rec[:], l0[:])
                bc = nrm_pool.tile([64, 1024], F32, name="bc", tag="bc")
                nc.gpsimd.partition_broadcast(bc[:], rec[0:1, :])
                ai, u = UNIT_SLOT[(b, mq)]
                ns = 512 // A2AS[ai][1]  # slots this unit's 512 cols span
                zq = [nc.sync, nc.gpsimd]  # parallel queues halve write latency
                for h in range(2):
                    zt = z_pool.tile([64, 512], BF16, name="ztile")
                    nc.vector.tensor_mul(
                        zt[:],
                        zcp[0:64, 512 * h:512 * h + 512],
                        bc[:, 512 * h:512 * h + 512],
                    )
                    if ns == 1:
                        zq[h].dma_start(
                            a2a_in[ai][u, 64 * h:64 * h + 64, :], zt[:]
                        )
                    else:
                        zq[h].dma_start(
                            a2a_in[ai][ns * u:ns * (u + 1), 64 * h:64 * h + 64, :]
                            .rearrange("r p c -> p r c"),
                            zt[:].rearrange("p (r c) -> p r c", r=ns),
                        )

            if defer:
                return fin, last_pe[0]
            fin()
            return None, last_pe[0]

        def emit_a2a(ai):
            nc.gpsimd.collective_compute(
                "AllToAll",
                mybir.AluOpType.bypass,
                replica_groups=[list(range(NCORES))],
                ins=[a2a_in[ai][:].opt()],
                outs=[a2a_out[ai][:].opt()],
            )

        # a cold collective pays ~12.5us between trigger and CC transfer;
        # ones fired within ~40us of a previous collective dispatch in ~2us.
        # This 256B dummy AllToAll, fired late in phase I, warms the CC pipe
        # so A2A_1 at the phase boundary dispatches fast.
        warm_in = dram.tile([NCORES, 16], BF16, name="warm_in")
        warm_out = dram.tile([NCORES, 16], BF16, name="warm_out")

        def emit_warm_a2a():
            t = wpool.tile([8, 16], BF16, name="ccwarm", tag="ccwarm")
            nc.gpsimd.memset(t[:], 0.0)
            nc.gpsimd.dma_start(warm_in[:], t[:])
            nc.gpsimd.collective_compute(
                "AllToAll",
                mybir.AluOpType.bypass,
                replica_groups=[list(range(NCORES))],
                ins=[warm_in[:].opt()],
                outs=[warm_out[:].opt()],
            )

        def emit_zrecv(ai, c):
            # one [128, 1024] tile = my output rows [128c, 128c+128) of this
            # exchange: col block c of all 8 source slots, side by side.
            t = zr_pool.tile([128, 1024], BF16, name=f"zr{ai}_{c}", tag="zr")
            nc.sync.dma_start(
                t[:].rearrange("p (s c) -> p s c", s=8),
                a2a_out[ai][:, :, 128 * c:128 * (c + 1)].rearrange("s p c -> p s c"),
            )
            zrt[(ai, c)] = t

        def emit_oproj_chunk(ai, c, after=None):
            # `after` pins the chunk's first matmul behind the intended
            # attention unit: the Tile scheduler otherwise hoists O-proj
            # (whose zrecv stationary load waits on a collective) ahead of
            # attention on the in-order PE queue, stalling it for the whole
            # collective (measured 49us).
            zr = zrt[(ai, c)]
            row0 = A2AS[ai][2] + 128 * c
            ob = ob_pool.tile([128, 1024], FP16, name="ob")
            pss = [
                ps_gen.tile([128, 512], F32, name="pso", tag="gen")
                for _ in range(2)
            ]
            first_mm = None
            for ct in range(8):
                for mh in range(2):
                    mm = nc.tensor.matmul(
                        pss[mh][:],
                        zr[:, 128 * ct:128 * (ct + 1)],
                        ot_sb[ct][:, 512 * mh:512 * (mh + 1)],
                        start=(ct == 0),
                        stop=(ct == 7),
                    )
                    if first_mm is None:
                        first_mm = mm
                        if after is not None:
                            add_dep_helper(
                                mm.ins,
                                after.ins,
                                sync=False,
                                reason="keep O-proj chunk at its emission slot",
                            )
            for mh in range(2):
                nc.vector.tensor_copy(ob[:, 512 * mh:512 * (mh + 1)], pss[mh][:])
            nc.gpsimd.dma_start(out_ext[row0:row0 + 128, :], ob[:])
            return first_mm

        # phase I: ALL projections + attention macros {1, 3}: proj (PE-only,
        # no exp) interleaves with the exp-heavy big macros, spreading the
        # 16MB x stream over the whole phase.
        pend = {}
        pend[(0, 0)] = emit_startup()
        pend[(0, 1)] = emit_xt(0, 1, three_way=True)
        emit_masks()
        fin = None
        for b in range(B):
            emit_proj(b, 0, pend.pop((b, 0)), fin=fin)
            fin = None
            emit_proj(b, 1, pend.pop((b, 1)))
            # batch 0's macro-2/3 loads precede any z write, so sync is
            # still safe as a third x queue for them
            pend[(b, 2)] = emit_xt(b, 2, three_way=(b == 0))
            pend[(b, 3)] = emit_xt(b, 3, three_way=(b == 0))
            f1, _ = emit_attn(b, 1, defer=True)
            if b + 1 < B:
                pend[(b + 1, 0)] = emit_xt(b + 1, 0)
            emit_proj(b, 2, pend.pop((b, 2)), fin=f1)
            if b + 1 < B:
                pend[(b + 1, 1)] = emit_xt(b + 1, 1)
            emit_proj(b, 3, pend.pop((b, 3)))
            if b == 3:
                emit_warm_a2a()
            fin, _ = emit_attn(b, 3, defer=True)
        fin()  # flush attn(3,3)'s tail: A2A_1 needs its z writes now
        emit_a2a(0)
        for m in range(8):
            nc.scalar.dma_start(ot_sb[m][:], ot_ext[128 * m:128 * (m + 1), :])
        # phase II: big (b,2) units first, tiny (b,0) units last. A2A_2 fires
        # after (3,2)'s deferred tail (inside U4); A2A_3 after (1,0)'s tail
        # (inside U6); A2A_4 after the final flush. The 4 chunks of A2A_1
        # land under U2..U5 (its ~25us cold latency from the boundary is over
        # by U2); A2A_2/3/4's chunks run back to back at the tail, keeping
        # the PE busy through A2A_4's latency.
        units2 = [(0, 2), (1, 2), (2, 2), (3, 2), (0, 0), (1, 0), (2, 0), (3, 0)]
        trig_plan = {3: 1}
        zrecv_plan = {1: [(0, 0), (0, 1)], 2: [(0, 2), (0, 3)], 6: [(1, 0)], 7: [(1, 1)]}
        chunk_plan = {3: (0, 0), 4: (0, 1), 5: (0, 2), 6: (0, 3), 7: (1, 0)}
        finII = None
        trig = None
        link = None
        for i, (bb, mqq) in enumerate(units2):
            fp, tg = finII, trig
            if fp is None and tg is None:
                fpw = None
            else:
                def fpw(fp=fp, tg=tg):
                    if fp is not None:
                        fp()
                    if tg is not None:
                        tg()
            finII, lpe = emit_attn(bb, mqq, fin_prev=fpw, defer=True)
            trig = (lambda a=trig_plan[i]: emit_a2a(a)) if i in trig_plan else None
            for (ai, c) in zrecv_plan.get(i, []):
                emit_zrecv(ai, c)
            if i in chunk_plan:
                ai, c = chunk_plan[i]
                link = emit_oproj_chunk(ai, c, after=(lpe or link))
        link = finII()  # flush attn(3,0)'s tail: A2A_3 needs its z now
        emit_a2a(2)
        link = emit_oproj_chunk(1, 1, after=link)
        emit_zrecv(2, 0)
        emit_zrecv(2, 1)
        link = emit_oproj_chunk(2, 0, after=link)
        emit_oproj_chunk(2, 1, after=link)

    nc.compile()
    return nc


_BUILT = {}


def _get_built(S):
    if S not in _BUILT:
        _BUILT[S] = build(S)
    return _BUILT[S]


def prep_inputs(x, Q, K, V, O):
    x = np.asarray(x, dtype=np.float32)
    Q = np.asarray(Q, dtype=np.float32)
    K = np.asarray(K, dtype=np.float32)
    V = np.asarray(V, dtype=np.float32)
    O = np.asarray(O, dtype=np.float32)
    xt = np.ascontiguousarray(np.transpose(x, (0, 2, 1))).astype(np.float16)  # [B, M, S]
    ot = np.ascontiguousarray(O.T).astype(ml_dtypes.bfloat16)  # [a, m], a = n*64+h
    in_maps = []
    for j in range(NCORES):
        hA, hB = 2 * j, 2 * j + 1
        wqk = np.ascontiguousarray(
            np.concatenate([Q[hA], Q[hB], K[hA], K[hB]], axis=0).T
        ).astype(np.float16)  # [1024, 256]
        wv = np.ascontiguousarray(
            np.concatenate([V[hA], V[hB]], axis=0).T
        ).astype(np.float16)  # [1024, 128]
        in_maps.append({"xt": xt, "wqk": wqk, "wv": wv, "ot": ot})
    return in_maps


def kernel(x, Q, K, V, O):
    global LAST_EXEC_TIME_NS
    x = np.asarray(x)
    S = x.shape[1]
    nc = _get_built(S)
    in_maps = prep_inputs(x, Q, K, V, O)
    trace = bool(int(os.environ.get("ATTN_TRACE", "0")))
    res = run_bass_kernel_spmd(nc, in_maps, list(range(NCORES)), trace=trace)
    LAST_EXEC_TIME_NS = res.exec_time_ns
    out = np.zeros((B, S, M), np.float32)
    # core j's out rows [base, base+W) of exchange ai = cols
    # [W*(j%cpu), +W) of unit units[j//cpu], cpu = 512//W.
    for j in range(NCORES):
        r = res.results[j]["out"]
        for units, W, base in A2AS:
            cpu = 512 // W
            b, mq = units[j // cpu]
            q0 = 512 * mq + W * (j % cpu)
            out[b, q0:q0 + W, :] = r[base:base + W, :]
    return out


# revision 24
# speedup vs baseline: 1.0199x; 1.0199x over previous
"""Distributed causal multi-head attention for 8 TRN2 NeuronCores.

Problem: x[4,2048,1024], per-head Q/K/V [16,64,1024], O [1024,1024].
  q,k,v = per-head projections of x; scores = q@k^T (no 1/sqrt(d));
  causal softmax; z = attn@v; out = z @ O^T.

Sharding (head-parallel): core j owns heads {2j, 2j+1} for ALL batches.
Per core:
  - x/Wq/Wk in fp16 (10-bit mantissa): scores are ~N(0, 64) with no 1/sqrt(d)
    scaling, so exp() amplifies absolute score error; bf16 inputs would give
    ~4% output error while fp16 gives ~0.5% and runs at full PE rate
    (f32r runs at half rate; f32 at quarter rate).
  - scoresT [k, q] layout: the softmax denominator comes for free from a
    ones-column appended to the PV stationary operand (psum row 64 = l);
    exp runs on ACT from 2-bank psum groups, psum -> sbuf bf16.
  - causal mask applied post-exp by multiplying with one of two STATIC mask
    tiles (built once at startup with gpsimd affine_select) as a single DVE
    tensor_mul per (head, diagonal k-pair). gpsimd-resident affine_select
    was ~2x slower per element and its queue backlog (broadcast/collective/
    out-writes) stalled PV by 1.5-4.5us per unit.
  - z is exchanged via FOUR AllToAlls. Measured collective cost model: a
    "cold" collective pays ~12.5us dispatch between its gpsimd trigger and
    the CC transfer (back-to-back collectives dispatch in ~2us), plus peer
    rendezvous equal to core arrival skew, plus ~1us/100KB transfer. Also
    z-write DMA completion gates the trigger, so z-write descriptor size
    matters (64-col routing = 128B descriptors measured ~5us per [64,512]
    write; full-unit slots give 1KB descriptors at ~1us). Hence:
      A2A_1: all 8 phase-I units {1,3}, slot = one full unit (512 cols,
             1KB descs), fired at the phase boundary; its 4 O-proj chunks
             land mid phase II.
      A2A_2: the 4 big phase-II units {(b,2)}, slot = half unit (256 cols,
             512B descs), fired right after (3,2)'s deferred tail, mid
             phase II; its 2 chunks land at the tail start.
      A2A_3/4: the tiny {(b,0)} units in pairs (slot 128 cols), fired as
             late pipeline stages; their latency is hidden under the 4
             deferred chunks of A2A_2/3 that fill the PE at the tail.
    Output rows are resharded accordingly (core j owns a W-wide column
    slice of each unit, W = 512/#cores-per-unit); kernel() unshards.
  - phase I = all projections + attention {1,3} (PE-bound: proj has no exp
    work to hide); phase II = {(0,2),(1,2),(2,2),(3,2),(0,0),(1,0),(2,0),
    (3,0)} - exp-heavy units first so the tail rides on the small ones.
    O-proj chunks are interleaved where the PE has slack, pinned by
    add_dep_helper anchors (the Tile scheduler otherwise hoists their
    zrecv-gated stationary loads ahead of attention on the in-order PE
    queue, stalling it for a whole collective - measured 49us).
  - zrecv reads are data-gated on a collective's completion semaphore and
    BLOCK the sync queue while waiting, delaying the z writes queued
    behind them (and hence the next collective): every zrecv is issued
    only where its collective is already (nearly) complete.
  - x feed: scalar+gpsimd queues; sync joins only for batch-0 units (it
    must stay clear of bulk traffic once softmax-gated z writes exist).
    wqk/x tile pairs interleave at startup so the first proj matmul needs
    only the first pair; a dummy exp warms the ACT table during the DMA
    wait (first real exp otherwise pays ~4.5us table load + ramp).
  - measured hazards baked in: ACT ops cost ~700ns fixed (never split exps);
    sub-128-row quadrant matmuls run ~1.5x slower (keep kz zero-padded);
    within a unit, scores(g+1) is emitted before PV(g) so the in-order PE
    never waits on exp(g); each unit's final PV + normalize are deferred
    into the next proj/unit.
"""

import os

import numpy as np
import ml_dtypes

import concourse.mybir as mybir
import concourse.tile as tile
from concourse.tile import add_dep_helper
from concourse import bacc
from concourse.bass_utils import run_bass_kernel_spmd

BF16 = mybir.dt.bfloat16
F32 = mybir.dt.float32
F32R = mybir.dt.float32r
FP16 = mybir.dt.float16

B, M, NH, DH = 4, 1024, 16, 64
NCORES = 8

# AllToAll groups: (units in completion order, slot width, out-row base).
# Slot width W = 512 * len(units) / 8; core j owns cols [W*(j%cpu), +W) of
# unit units[j//cpu] where cpu = 512//W cores share a unit.
A2AS = [
    ([(0, 1), (0, 3), (1, 1), (1, 3), (2, 1), (2, 3), (3, 1), (3, 3)], 512, 0),
    ([(0, 2), (1, 2), (2, 2), (3, 2)], 256, 512),
    ([(0, 0), (1, 0), (2, 0), (3, 0)], 256, 768),
]
UNIT_SLOT = {}
for _ai, (_units, _w, _base) in enumerate(A2AS):
    for _u, _bm in enumerate(_units):
        UNIT_SLOT[_bm] = (_ai, _u)

LAST_EXEC_TIME_NS = None


def build(S=2048):
    GQ = B * S
    CH = GQ // NCORES      # output rows per core

    nc = bacc.Bacc("TRN2", target_bir_lowering=False, debug=False, num_devices=NCORES)
    xt_ext = nc.dram_tensor("xt", [B, M, S], FP16, kind="ExternalInput")
    wqk_ext = nc.dram_tensor("wqk", [M, 256], FP16, kind="ExternalInput")
    wv_ext = nc.dram_tensor("wv", [M, 128], FP16, kind="ExternalInput")
    ot_ext = nc.dram_tensor("ot", [M, M], BF16, kind="ExternalInput")
    # fp16 output (~5e-4 rounding, well within budget) halves the tail
    # out-write traffic; kernel() casts back to f32.
    out_ext = nc.dram_tensor("out", [CH, M], FP16, kind="ExternalOutput")

    Exp = mybir.ActivationFunctionType.Exp

    with (
        tile.TileContext(nc) as tc,
        tc.tile_pool(name="wpool", bufs=1) as wpool,
        tc.tile_pool(name="xt", bufs=32) as xt_pool,
        tc.tile_pool(name="qk", bufs=1) as qk_pool,
        tc.tile_pool(name="kz", bufs=1) as kz_pool,
        tc.tile_pool(name="vp", bufs=1) as v_pool,
        tc.tile_pool(name="ep", bufs=7) as e_pool,
        tc.tile_pool(name="zp", bufs=12) as z_pool,
        tc.tile_pool(name="zr", bufs=5) as zr_pool,
        tc.tile_pool(name="ob", bufs=2) as ob_pool,
        tc.tile_pool(name="nrm", bufs=2) as nrm_pool,
        tc.tile_pool(name="ps_sc", bufs=2, space="PSUM") as ps_sc,
        tc.tile_pool(name="ps_z", bufs=1, space="PSUM") as ps_z,
        tc.tile_pool(name="ps_gen", bufs=2, space="PSUM") as ps_gen,
        tc.tile_pool(name="dram", bufs=1, space="DRAM") as dram,
    ):
        xq3 = [nc.scalar, nc.gpsimd, nc.sync]
        xq2 = [nc.scalar, nc.gpsimd]
        wqk_sb, wv_sb, ot_sb = [], [], []
        for m in range(8):
            wqk_sb.append(wpool.tile([128, 256], FP16, name=f"wqk{m}", tag=f"wqk{m}"))
            wv_sb.append(wpool.tile([128, 128], FP16, name=f"wv{m}", tag=f"wv{m}"))
            ot_sb.append(wpool.tile([128, 1024], BF16, name=f"ot{m}", tag=f"ot{m}"))

        a2a_in = [
            dram.tile([NCORES, 128, w], BF16, name=f"a2a_in{ai}")
            for ai, (_, w, _b) in enumerate(A2AS)
        ]
        a2a_out = [
            dram.tile([NCORES, 128, w], BF16, name=f"a2a_out{ai}")
            for ai, (_, w, _b) in enumerate(A2AS)
        ]

        # static causal masks for the two diagonal k-tile pairs of any unit:
        # mask[kpos, 512*kk + q] = 1 if q >= kpos + 128*(d0+kk) else 0.
        # Built AFTER the startup x DMAs are issued (emit_masks call below):
        # the memset/affine ops share the gpsimd queue with x-tile issues and
        # cost ~3.5us of first-tile delay if emitted first.
        masks = {}

        def emit_masks():
            for d0 in (0, 2):
                t = wpool.tile([128, 1024], BF16, name=f"mask{d0}", tag=f"mask{d0}")
                nc.gpsimd.memset(t[:], 1.0)
                for kk in range(2):
                    sl = t[:, 512 * kk:512 * (kk + 1)]
                    nc.gpsimd.affine_select(
                        out=sl,
                        in_=sl,
                        compare_op=mybir.AluOpType.is_ge,
                        fill=0.0,
                        base=-128 * (d0 + kk),
                        pattern=[[1, 512]],
                        channel_multiplier=-1,
                    )
                masks[d0] = t

        qk_sb = {}   # (ct, b, mq) -> [128, 512] fp16; ct0 = qT (2 heads)
        kz_sb = {}   # (h, b, mk) -> [128, 512] fp16 zero-padded per-head kT
        v_sb = {}    # (b, k_tile) -> [128, 130] bf16: [vA(64) | 1 | vB(64) | 1]
        zrt = {}     # (ai, c) -> [128, 1024] bf16 zrecv tile

        def emit_xt(b, mq, three_way=False):
            qs = xq3 if three_way else xq2
            xts = []
            for m in range(8):
                t = xt_pool.tile([128, 512], FP16, name="xtc")
                qs[m % len(qs)].dma_start(
                    t[:], xt_ext[b, 128 * m:128 * (m + 1), 512 * mq:512 * (mq + 1)]
                )
                xts.append(t)
            return xts

        def emit_startup():
            # wqk/x pairs: queue m%3 gets wqk_m then x(0,0)_m back to back, so
            # the m=0 pair (all the first matmul needs) lands first on scalar;
            # a zeroed dummy exp loads the ACT table during the DMA wait.
            xts0 = []
            for m in range(8):
                q = xq3[m % 3]
                q.dma_start(wqk_sb[m][:], wqk_ext[128 * m:128 * (m + 1), :])
                t = xt_pool.tile([128, 512], FP16, name="xtc")
                q.dma_start(t[:], xt_ext[0, 128 * m:128 * (m + 1), 0:512])
                xts0.append(t)
            warm = wpool.tile([1, 16], F32, name="actwarm", tag="actwarm")
            nc.gpsimd.memset(warm[:], 0.0)
            nc.scalar.activation(warm[:], warm[:], Exp)
            for m in range(8):
                xq3[m % 3].dma_start(wv_sb[m][:], wv_ext[128 * m:128 * (m + 1), :])
            return xts0

        def emit_proj(b, mq, xts, fin=None):
            for ct in range(2):
                ps = ps_gen.tile([128, 512], F32, name="psqk", tag="gen")
                for m in range(8):
                    nc.tensor.matmul(
                        ps[:],
                        wqk_sb[m][:, 128 * ct:128 * (ct + 1)],
                        xts[m][:],
                        start=(m == 0),
                        stop=(m == 7),
                    )
                if ct == 0:
                    t = qk_pool.tile(
                        [128, 512], FP16, name=f"qk{ct}_{b}_{mq}", tag=f"qk{ct}_{b}_{mq}"
                    )
                    nc.vector.tensor_copy(t[:], ps[:])
                    qk_sb[(ct, b, mq)] = t
                    # the previous attention unit's deferred tail (final PV +
                    # normalize) lands here: the ct0 matmuls above fill the
                    # PE while that unit's last exp drains on ACT
                    if fin is not None:
                        fin()
                else:
                    # kT is consumed only as zero-padded per-head copies:
                    # K=128 scores matmuls run at full rate while 64-row
                    # quadrant matmuls measured ~1.5x slower per instruction.
                    # Copies on DVE: ACT is the attention pacer (exp), keep
                    # it exp-only.
                    for h in range(2):
                        kz = kz_pool.tile(
                            [128, 512], FP16, name=f"kz{h}_{b}_{mq}",
                            tag=f"kz{h}_{b}_{mq}",
                        )
                        nc.vector.memset(kz[64 - 64 * h:128 - 64 * h, :], 0.0)
                        nc.vector.tensor_copy(
                            kz[64 * h:64 * (h + 1), :],
                            ps[64 * h:64 * (h + 1), :],
                        )
                        kz_sb[(h, b, mq)] = kz
            for stl in range(4):
                ps = ps_gen.tile([128, 128], F32, name="psv", tag="gen")
                for m in range(8):
                    nc.tensor.matmul(
                        ps[:],
                        xts[m][:, 128 * stl:128 * (stl + 1)],
                        wv_sb[m][:],
                        start=(m == 0),
                        stop=(m == 7),
                    )
                kt = 4 * mq + stl
                # layout [vA(64) | 1 | vB(64) | 1]: ones column makes PV row 64
                # the softmax denominator; z dims land on rows 0..63 (DVE
                # partition ranges must start at 0/32/64/96, so z-rows-first)
                vt = v_pool.tile([128, 130], BF16, name=f"v_{b}_{kt}", tag=f"v_{b}_{kt}")
                nc.gpsimd.memset(vt[:, 64:65], 1.0)
                nc.gpsimd.memset(vt[:, 129:130], 1.0)
                nc.vector.tensor_copy(
                    vt[:].rearrange("p (g c) -> p g c", g=2)[:, :, 0:64],
                    ps[:].rearrange("p (g c) -> p g c", g=2),
                )
                v_sb[(b, kt)] = vt

        def emit_attn(b, mq, fin_prev=None, defer=False):
            nk = 4 * (mq + 1)
            # pz is allocated lazily at the first PV so pool-slot WAR
            # tracking stays consistent with deferred tails (the previous
            # unit's final PV may be emitted after this unit starts)
            pzc = []

            def get_pz():
                if not pzc:
                    pzc.append(ps_z.tile([128, 1024], F32, name="pz", tag="pz"))
                return pzc[0]

            def emit_scores_exp(g):
                # per head: 2 scores matmuls then immediately the exp, so ACT
                # starts on head 0 while the PE does head 1's scores. One
                # full-width exp per (group, head): ACT instructions have
                # ~700ns fixed overhead, so fewer/wider beats masked-region
                # skipping (measured +24us ACT when split per ktile).
                es = []
                for h in range(2):
                    psc = ps_sc.tile([128, 1024], F32, name="psc", tag="sc")
                    for kk in range(2):
                        kt = 2 * g + kk
                        mk, ktl = kt // 4, kt % 4
                        # diagonal tiles: q columns < 128*d are fully masked,
                        # so skip them in both scores and PV. The psum left
                        # unwritten holds stale-but-finite values whose exp
                        # is zeroed by the mask multiply (q < 128d => masked).
                        d = kt - 4 * mq
                        q0 = 128 * d if d > 0 else 0
                        nc.tensor.matmul(
                            psc[:, 512 * kk + q0:512 * (kk + 1)],
                            kz_sb[(h, b, mk)][:, 128 * ktl:128 * (ktl + 1)],
                            qk_sb[(0, b, mq)][:, q0:512],
                            start=True,
                            stop=True,
                        )
                    e = e_pool.tile([128, 1024], BF16, name="etile")
                    nc.scalar.activation(e[:], psc[:], Exp)
                    es.append(e)
                d0 = 2 * g - 4 * mq
                if d0 >= 0:  # diagonal pair: zero where k + 128*d > q
                    for h in range(2):
                        nc.vector.tensor_mul(es[h][:], es[h][:], masks[d0][:])
                return es

            last_pe = [None]

            def emit_pv(g, es):
                pz = get_pz()
                for kk in range(2):
                    kt = 2 * g + kk
                    d = kt - 4 * mq
                    q0 = 128 * d if d > 0 else 0
                    vt = v_sb[(b, kt)]
                    for h in range(2):
                        last_pe[0] = nc.tensor.matmul(
                            pz[0:65, 512 * h + q0:512 * h + 512],
                            vt[:, 65 * h:65 * h + 65],
                            es[h][:, 512 * kk + q0:512 * (kk + 1)],
                            start=(kt == 0),
                            stop=(kt == nk - 1),
                        )

            # software pipeline: scores(g+1) is emitted BEFORE PV(g). The PE
            # queue is in-order, so otherwise PV(g)'s wait on exp(g) blocks
            # scores(g+1) that could already run - ~1us/group of PE idle.
            # The final PV + normalize are deferred (fin) so the NEXT
            # proj/unit's first matmuls can fill the last exp's latency; the
            # previous unit's fin lands right after this unit's first group.
            prev = None
            first = True
            for g in range(nk // 2):
                es = emit_scores_exp(g)
                if first:
                    if fin_prev is not None:
                        fin_prev()
                    first = False
                if prev is not None:
                    emit_pv(prev[0], prev[1])
                prev = (g, es)

            def fin():
                emit_tail(prev)
                return last_pe[0]

            def emit_tail(prev):
                emit_pv(prev[0], prev[1])
                pz = get_pz()
                # normalize: pz row 64 of each half = l. partition_broadcast
                # only reads from base partition 0, and DVE can't shift
                # partitions, so DMA the l row from psum partition 64 to
                # sbuf partition 0 first. The l row is copied FIRST (tiny DVE
                # op) so the l0-dma/reciprocal/broadcast chain starts ~1.5us
                # before the bulk z copy lands - this chain gates the z
                # writes and hence the collective triggers.
                zcp = nrm_pool.tile([65, 1024], F32, name="zcp", tag="zcp")
                nc.vector.tensor_copy(zcp[64:65, :], pz[64:65, :])
                l0 = nrm_pool.tile([1, 1024], F32, name="l0", tag="l0")
                nc.gpsimd.dma_start(l0[:], zcp[64:65, :])
                nc.vector.tensor_copy(zcp[0:64, 0:512], pz[0:64, 0:512])
                nc.scalar.activation(
                    zcp[0:64, 512:1024],
                    pz[0:64, 512:1024],
                    mybir.ActivationFunctionType.Copy,
                )
                rec = nrm_pool.tile([1, 1024], F32, name="rec", tag="rec")
                nc.vector.reciprocal_approx_fast(rec[:], l0[:])
                bc = nrm_pool.tile([64, 1024], F32, name="bc", tag="bc")
                nc.gpsimd.partition_broadcast(bc[:], rec[0:1, :])
                ai, u = UNIT_SLOT[(b, mq)]
                ns = 512 // A2AS[ai][1]  # slots this unit's 512 cols span
                # both on sync: gpsimd/scalar DMA issues here block the x feed
                # or the exp pacer behind softmax-gated data (measured -17us)
                zq = [nc.sync, nc.sync]
                for h in range(2):
                    zt = z_pool.tile([64, 512], BF16, name="ztile")
                    nc.vector.tensor_mul(
                        zt[:],
                        zcp[0:64, 512 * h:512 * h + 512],
                        bc[:, 512 * h:512 * h + 512],
                    )
                    if ns == 1:
                        zq[h].dma_start(
                            a2a_in[ai][u, 64 * h:64 * h + 64, :], zt[:]
                        )
                    else:
                        zq[h].dma_start(
                            a2a_in[ai][ns * u:ns * (u + 1), 64 * h:64 * h + 64, :]
                            .rearrange("r p c -> p r c"),
                            zt[:].rearrange("p (r c) -> p r c", r=ns),
                        )

            if defer:
                return fin, last_pe[0]
            fin()
            return None, last_pe[0]

        def emit_a2a(ai):
            nc.gpsimd.collective_compute(
                "AllToAll",
                mybir.AluOpType.bypass,
                replica_groups=[list(range(NCORES))],
                ins=[a2a_in[ai][:].opt()],
                outs=[a2a_out[ai][:].opt()],
            )

        # a cold collective pays ~12.5us between trigger and CC transfer;
        # ones fired within ~40us of a previous collective dispatch in ~2us.
        # This 256B dummy AllToAll, fired late in phase I, warms the CC pipe
        # so A2A_1 at the phase boundary dispatches fast.
        warm_in = dram.tile([NCORES, 16], BF16, name="warm_in")
        warm_out = dram.tile([NCORES, 16], BF16, name="warm_out")

        def emit_warm_a2a():
            t = wpool.tile([8, 16], BF16, name="ccwarm", tag="ccwarm")
            nc.gpsimd.memset(t[:], 0.0)
            nc.gpsimd.dma_start(warm_in[:], t[:])
            nc.gpsimd.collective_compute(
                "AllToAll",
                mybir.AluOpType.bypass,
                replica_groups=[list(range(NCORES))],
                ins=[warm_in[:].opt()],
                outs=[warm_out[:].opt()],
            )

        def emit_zrecv(ai, c):
            # one [128, 1024] tile = my output rows [128c, 128c+128) of this
            # exchange: col block c of all 8 source slots, side by side.
            t = zr_pool.tile([128, 1024], BF16, name=f"zr{ai}_{c}", tag="zr")
            nc.sync.dma_start(
                t[:].rearrange("p (s c) -> p s c", s=8),
                a2a_out[ai][:, :, 128 * c:128 * (c + 1)].rearrange("s p c -> p s c"),
            )
            zrt[(ai, c)] = t

        def emit_oproj_chunk(ai, c, after=None):
            # `after` pins the chunk's first matmul behind the intended
            # attention unit: the Tile scheduler otherwise hoists O-proj
            # (whose zrecv stationary load waits on a collective) ahead of
            # attention on the in-order PE queue, stalling it for the whole
            # collective (measured 49us).
            zr = zrt[(ai, c)]
            row0 = A2AS[ai][2] + 128 * c
            ob = ob_pool.tile([128, 1024], FP16, name="ob")
            pss = [
                ps_gen.tile([128, 512], F32, name="pso", tag="gen")
                for _ in range(2)
            ]
            first_mm = None
            for ct in range(8):
                for mh in range(2):
                    mm = nc.tensor.matmul(
                        pss[mh][:],
                        zr[:, 128 * ct:128 * (ct + 1)],
                        ot_sb[ct][:, 512 * mh:512 * (mh + 1)],
                        start=(ct == 0),
                        stop=(ct == 7),
                    )
                    if first_mm is None:
                        first_mm = mm
                        if after is not None:
                            add_dep_helper(
                                mm.ins,
                                after.ins,
                                sync=False,
                                reason="keep O-proj chunk at its emission slot",
                            )
            for mh in range(2):
                nc.vector.tensor_copy(ob[:, 512 * mh:512 * (mh + 1)], pss[mh][:])
            nc.gpsimd.dma_start(out_ext[row0:row0 + 128, :], ob[:])
            return first_mm

        # phase I: ALL projections + attention macros {1, 3}: proj (PE-only,
        # no exp) interleaves with the exp-heavy big macros, spreading the
        # 16MB x stream over the whole phase.
        pend = {}
        pend[(0, 0)] = emit_startup()
        pend[(0, 1)] = emit_xt(0, 1, three_way=True)
        emit_masks()
        fin = None
        for b in range(B):
            emit_proj(b, 0, pend.pop((b, 0)), fin=fin)
            fin = None
            emit_proj(b, 1, pend.pop((b, 1)))
            # batch 0's macro-2/3 loads precede any z write, so sync is
            # still safe as a third x queue for them
            pend[(b, 2)] = emit_xt(b, 2, three_way=(b == 0))
            pend[(b, 3)] = emit_xt(b, 3, three_way=(b == 0))
            f1, _ = emit_attn(b, 1, defer=True)
            if b + 1 < B:
                pend[(b + 1, 0)] = emit_xt(b + 1, 0)
            emit_proj(b, 2, pend.pop((b, 2)), fin=f1)
            if b + 1 < B:
                pend[(b + 1, 1)] = emit_xt(b + 1, 1)
            emit_proj(b, 3, pend.pop((b, 3)))
            if b == 3:
                emit_warm_a2a()
            fin, _ = emit_attn(b, 3, defer=True)
        fin()  # flush attn(3,3)'s tail: A2A_1 needs its z writes now
        emit_a2a(0)
        for m in range(8):
            nc.scalar.dma_start(ot_sb[m][:], ot_ext[128 * m:128 * (m + 1), :])
        # phase II: big (b,2) units first, tiny (b,0) units last. A2A_2 fires
        # after (3,2)'s deferred tail (inside U4); A2A_3 after (1,0)'s tail
        # (inside U6); A2A_4 after the final flush. The 4 chunks of A2A_1
        # land under U2..U5 (its ~25us cold latency from the boundary is over
        # by U2); A2A_2/3/4's chunks run back to back at the tail, keeping
        # the PE busy through A2A_4's latency.
        units2 = [(0, 2), (1, 2), (2, 2), (3, 2), (0, 0), (1, 0), (2, 0), (3, 0)]
        trig_plan = {3: 1}
        zrecv_plan = {1: [(0, 0), (0, 1)], 2: [(0, 2), (0, 3)], 6: [(1, 0)], 7: [(1, 1)]}
        chunk_plan = {3: (0, 0), 4: (0, 1), 5: (0, 2), 6: (0, 3)}
        finII = None
        trig = None
        link = None
        for i, (bb, mqq) in enumerate(units2):
            fp, tg = finII, trig
            if fp is None and tg is None:
                fpw = None
            else:
                def fpw(fp=fp, tg=tg):
                    if fp is not None:
                        fp()
                    if tg is not None:
                        tg()
            finII, lpe = emit_attn(bb, mqq, fin_prev=fpw, defer=True)
            trig = (lambda a=trig_plan[i]: emit_a2a(a)) if i in trig_plan else None
            for (ai, c) in zrecv_plan.get(i, []):
                emit_zrecv(ai, c)
            if i in chunk_plan:
                ai, c = chunk_plan[i]
                link = emit_oproj_chunk(ai, c, after=(lpe or link))
        # flush BEFORE any remaining chunk: the last unit's normalize chain
        # (l0/broadcast on Pool, z writes on sync) must run ahead of chunk
        # out-writes, else the final trigger waits on the whole chunk
        # pipeline (measured 28us of trigger delay). The 4 chunks below then
        # fill the PE exactly during A2A_3's ~18us latency.
        link = finII()
        emit_a2a(2)
        link = emit_oproj_chunk(1, 0, after=link)
        link = emit_oproj_chunk(1, 1, after=link)
        emit_zrecv(2, 0)
        emit_zrecv(2, 1)
        link = emit_oproj_chunk(2, 0, after=link)
        emit_oproj_chunk(2, 1, after=link)

    nc.compile()
    return nc


_BUILT = {}


def _get_built(S):
    if S not in _BUILT:
        _BUILT[S] = build(S)
    return _BUILT[S]


def prep_inputs(x, Q, K, V, O):
    x = np.asarray(x, dtype=np.float32)
    Q = np.asarray(Q, dtype=np.float32)
    K = np.asarray(K, dtype=np.float32)
    V = np.asarray(V, dtype=np.float32)
    O = np.asarray(O, dtype=np.float32)
    xt = np.ascontiguousarray(np.transpose(x, (0, 2, 1))).astype(np.float16)  # [B, M, S]
    ot = np.ascontiguousarray(O.T).astype(ml_dtypes.bfloat16)  # [a, m], a = n*64+h
    in_maps = []
    for j in range(NCORES):
        hA, hB = 2 * j, 2 * j + 1
        wqk = np.ascontiguousarray(
            np.concatenate([Q[hA], Q[hB], K[hA], K[hB]], axis=0).T
        ).astype(np.float16)  # [1024, 256]
        wv = np.ascontiguousarray(
            np.concatenate([V[hA], V[hB]], axis=0).T
        ).astype(np.float16)  # [1024, 128]
        in_maps.append({"xt": xt, "wqk": wqk, "wv": wv, "ot": ot})
    return in_maps


def kernel(x, Q, K, V, O):
    global LAST_EXEC_TIME_NS
    x = np.asarray(x)
    S = x.shape[1]
    nc = _get_built(S)
    in_maps = prep_inputs(x, Q, K, V, O)
    trace = bool(int(os.environ.get("ATTN_TRACE", "0")))
    res = run_bass_kernel_spmd(nc, in_maps, list(range(NCORES)), trace=trace)
    LAST_EXEC_TIME_NS = res.exec_time_ns
    out = np.zeros((B, S, M), np.float32)
    # core j's out rows [base, base+W) of exchange ai = cols
    # [W*(j%cpu), +W) of unit units[j//cpu], cpu = 512//W.
    for j in range(NCORES):
        r = res.results[j]["out"]
        for units, W, base in A2AS:
            cpu = 512 // W
            b, mq = units[j // cpu]
            q0 = 512 * mq + W * (j % cpu)
            out[b, q0:q0 + W, :] = r[base:base + W, :]
    return out
